# revision 1
# baseline (speedup 1.0000x reference)
"""ApproachLoss kernel for 8 TRN2 NeuronCores (Bass/Tile).

Reference computation (per batch element b):
    deltas[t]  = ||states[b, t+1] - states[b, t]||          t in [0, L-2]
    di[j]      = relu(deltas[j+1] - deltas[j])              j in [0, L-3]
    weighted   = di * reasoning_mask[b, 2:] * approach_weight
    loss       = sum_b sum_j weighted / (sum_b sum_t mask[b, 2:] + 1e-9)

Sharding: pure data-parallel, batch element b -> core b. Each core returns
[weighted_sum_b, mask_sum_b]; the host sums the 16 scalars and divides.

Device layout per core (states shard [4096, 1024] f32, ~16.8 MB -> the
kernel is HBM-DMA bound at ~358 GB/s/core, floor ~47 us/core):
  33 token windows of 128 tokens with stride 127 (1-token overlap), so
  every adjacent-token pair lands inside some window. Per window:
    DMA [128, 1024] to SBUF as bf16 (even windows: SWDGE DMA with inline
    f32->bf16 cast; odd windows: HWDGE f32 DMA + VectorE cast — the
    split uses both descriptor-generation paths and halves the SWDGE
    ring load that slows SDMA engines 7/15)
    -> TensorE shift-difference matmul (stationary W, W[j, j] = -1,
       W[j+1, j] = +1, exact in bf16): PSUM[j, d] = x[j+1, d] - x[j, d]
    -> ScalarE activation(Square, accum_out): fused square + reduce over
       D in one pass; column i of R[128, 33] gets ssq at delta index
       t = 127*i + p.
  Tail: E = sqrt(R) -> delta_increase via the same shift matmul plus a
  second accumulating matmul (wfix[0, 126] = 1 against column-shifted E)
  for the window-boundary terms -> relu -> * host-precomputed
  (mask*weight) tile -> free-dim reduce -> ones-matmul partition
  reduction -> out [1, 2] = [weighted_sum, mask_sum].

Measured: ~64-70 us HW exec (best 63.6; run-to-run HBM/scheduling noise
~+-4 us); ~6 us fixed NEFF startup + ~4 us Tile end-barrier included.
DMA roofline alone is ~47 us/core.
"""

import ml_dtypes
import numpy as np

B, L, D = 8, 4096, 1024
NT = 33          # diff tiles: tile i covers tokens 127i .. 127i+127
STRIDE = 127     # valid diffs per full tile
N_CORES = 8

_CACHE = {}


def _build_nc(split_mode="act_act"):
    import concourse.bass as bass  # noqa: F401
    import concourse.tile as tile
    from concourse import bacc, mybir

    f32 = mybir.dt.float32
    nc = bacc.Bacc(
        "TRN2", target_bir_lowering=False, debug=False, num_devices=N_CORES
    )

    states = nc.declare_dram_parameter("states", [L, D], f32, isOutput=False)
    wshift = nc.declare_dram_parameter("wshift", [128, 128], f32, isOutput=False)
    wfix = nc.declare_dram_parameter("wfix", [128, 128], f32, isOutput=False)
    wbf = nc.declare_dram_parameter(
        "wbf", [128, 128], mybir.dt.bfloat16, isOutput=False
    )
    mw = nc.declare_dram_parameter("mw", [128, NT], f32, isOutput=False)
    maskt = nc.declare_dram_parameter("maskt", [128, 32], f32, isOutput=False)
    ones = nc.declare_dram_parameter("ones", [128, 1], f32, isOutput=False)
    out = nc.declare_dram_parameter("out", [1, 2], f32, isOutput=True)

    with tile.TileContext(nc) as tc:
        bf16 = mybir.dt.bfloat16
        with (
            tc.tile_pool(name="consts", bufs=1) as consts,
            tc.tile_pool(name="xbpool", bufs=8) as xbpool,
            tc.tile_pool(name="xpool", bufs=4) as xpool,
            tc.tile_pool(name="scr", bufs=2) as scrpool,
            tc.tile_pool(name="psum", bufs=3, space="PSUM") as pspool,
            tc.tile_pool(name="psmall", bufs=1, space="PSUM") as psmall,
        ):
            w_bf = consts.tile([128, 128], bf16)
            nc.sync.dma_start(out=w_bf, in_=wbf[:, :])
            w_sb = consts.tile([128, 128], f32)
            nc.sync.dma_start(out=w_sb, in_=wshift[:, :])
            wfix_sb = consts.tile([128, 128], f32)
            nc.sync.dma_start(out=wfix_sb, in_=wfix[:, :])
            mw_sb = consts.tile([128, NT], f32)
            nc.sync.dma_start(out=mw_sb, in_=mw[:, :])
            mask_sb = consts.tile([128, 32], f32)
            nc.sync.dma_start(out=mask_sb, in_=maskt[:, :])
            ones_sb = consts.tile([128, 1], f32)
            nc.sync.dma_start(out=ones_sb, in_=ones[:, :])

            # dummy sqrt up front: forces the ACT table pass to load
            # sqrt_and_others (which also contains square) once, instead of
            # a second serial table load in the tail
            warm = consts.tile([1, 1], f32)
            nc.scalar.sqrt(warm, ones_sb[0:1, 0:1])

            r_a = consts.tile([128, NT], f32)
            e_sb = consts.tile([128, NT + 1], f32)
            nc.vector.memset(e_sb[:, NT : NT + 1], 0.0)
            g = consts.tile([128, 2], f32)
            nc.vector.tensor_reduce(
                g[:, 1:2], mask_sb, axis=mybir.AxisListType.X, op=mybir.AluOpType.add
            )

            # Main loop over 33 token windows (stride 127, 1-token overlap).
            # bf16 tiles for full-rate matmul (shift weights are +-1, exact
            # in bf16; states rounding costs ~1e-4 on the loss). Alternate
            # windows between SWDGE DMA with inline f32->bf16 cast and
            # HWDGE f32 DMA + VectorE cast: splits traffic across both DGE
            # paths and halves the SWDGE descriptor-ring load that slows
            # SDMA engines 7/15.
            # process the small partial window first: its DMA completes
            # ~4x sooner, priming the matmul/square pipeline earlier
            for i in [NT - 1] + list(range(NT - 1)):
                rows = 128 if i < NT - 1 else L - STRIDE * (NT - 1)
                xb = xbpool.tile([128, D], bf16)
                if rows < 128:
                    nc.vector.memset(xb, 0.0)
                if i % 2 == 0:
                    nc.gpsimd.dma_start(
                        out=xb[0:rows, :],
                        in_=states[STRIDE * i : STRIDE * i + rows, :],
                    )
                else:
                    x = xpool.tile([128, D], f32)
                    nc.sync.dma_start(
                        out=x[0:rows, :],
                        in_=states[STRIDE * i : STRIDE * i + rows, :],
                    )
                    nc.vector.tensor_copy(xb[0:rows, :], x[0:rows, :])

                ps = pspool.tile([128, D], f32)
                nc.tensor.matmul(
                    ps[:, 0:512], lhsT=w_bf, rhs=xb[:, 0:512], start=True, stop=True
                )
                nc.tensor.matmul(
                    ps[:, 512:1024], lhsT=w_bf, rhs=xb[:, 512:1024],
                    start=True, stop=True,
                )

                # ScalarE fused square + free-dim accumulate over both banks
                scr = scrpool.tile([128, D], f32)
                nc.scalar.activation(
                    scr,
                    ps,
                    mybir.ActivationFunctionType.Square,
                    accum_out=r_a[:, i : i + 1],
                )

            # ---- tail: E = sqrt(R) (padded with a zero column) ----
            nc.scalar.activation(
                e_sb[:, 0:NT], r_a, mybir.ActivationFunctionType.Sqrt
            )

            # psD[j, i] = E[j+1, i] - E[j, i]; row 126 needs E[0, i+1]
            # (delta at the tile boundary) — added by a second accumulating
            # matmul with wfix[0, 126] = 1 against the column-shifted E.
            # E[127, :] is all zeros so the first matmul contributes only
            # -E[126, i] to row 126.
            ps_d = psmall.tile([128, NT], f32)
            nc.tensor.matmul(
                ps_d, lhsT=w_sb, rhs=e_sb[:, 0:NT], start=True, stop=False
            )
            nc.tensor.matmul(
                ps_d,
                lhsT=wfix_sb,
                rhs=e_sb[:, 1 : NT + 1],
                start=False,
                stop=True,
            )

            # fused relu(psD) * mw + free-dim reduce in one DVE op
            wt = consts.tile([128, NT], f32)
            nc.vector.scalar_tensor_tensor(
                out=wt,
                in0=ps_d,
                scalar=0.0,
                in1=mw_sb,
                op0=mybir.AluOpType.max,
                op1=mybir.AluOpType.mult,
                accum_out=g[:, 0:1],
            )

            ps_s = psmall.tile([1, 2], f32)
            nc.tensor.matmul(ps_s, lhsT=ones_sb, rhs=g, start=True, stop=True)
            out_sb = consts.tile([1, 2], f32)
            nc.vector.tensor_copy(out_sb, ps_s)
            nc.sync.dma_start(out=out[:, :], in_=out_sb)

    nc.compile()
    return nc


def _host_consts():
    w = np.zeros((128, 128), dtype=np.float32)
    for j in range(127):
        w[j, j] = -1.0
        w[j + 1, j] = 1.0
    wfix = np.zeros((128, 128), dtype=np.float32)
    wfix[0, 126] = 1.0
    ones = np.ones((128, 1), dtype=np.float32)
    return w, wfix, ones


def _per_core_inputs(states_b, mask_b, rp_b, w, wfix, ones):
    # weighted-sum coefficients: mw[p, i] = mask[t+2] * weight[t], t = 127i+p
    t = np.arange(L - 2, dtype=np.float64)
    dist = np.maximum(float(rp_b) - t - 2.0, 0.0)
    weight = np.where(dist < 5, 2.0 + (5.0 - dist) * 0.5, 1.0).astype(np.float32)
    mwvec = (mask_b[2:L] * weight).astype(np.float32)  # [L-2]
    vals = np.zeros(NT * STRIDE, dtype=np.float32)
    vals[: L - 2] = mwvec
    mw = np.zeros((128, NT), dtype=np.float32)
    mw[:STRIDE, :] = vals.reshape(NT, STRIDE).T

    mt = mask_b.astype(np.float32).copy()
    mt[0:2] = 0.0
    maskt = mt.reshape(128, 32)

    return {
        "states": np.ascontiguousarray(states_b, dtype=np.float32),
        "wshift": w,
        "wfix": wfix,
        "wbf": w.astype(ml_dtypes.bfloat16),
        "mw": mw,
        "maskt": np.ascontiguousarray(maskt),
        "ones": ones,
    }


def _get_nc(split_mode="act_act"):
    key = ("nc", split_mode)
    if key not in _CACHE:
        _CACHE[key] = _build_nc(split_mode)
    return _CACHE[key]


def _run(states, reasoning_mask, result_token_positions, trace=False,
         split_mode="act_act"):
    from concourse.bass_utils import run_bass_kernel_spmd

    states = np.asarray(states, dtype=np.float32)
    mask = np.asarray(reasoning_mask, dtype=np.float32)
    rp = np.asarray(result_token_positions)

    w, wfix, ones = _host_consts()
    in_maps = [
        _per_core_inputs(states[b], mask[b], rp[b], w, wfix, ones)
        for b in range(N_CORES)
    ]
    nc = _get_nc(split_mode)
    res = run_bass_kernel_spmd(
        nc, in_maps, core_ids=list(range(N_CORES)), trace=trace
    )
    partials = np.stack([res.results[i]["out"][0] for i in range(N_CORES)])  # [8, 2]
    s = partials[:, 0].astype(np.float64).sum()
    m = partials[:, 1].astype(np.float64).sum()
    value = np.float32(s / (m + 1e-9))
    return value, res


def kernel(states, reasoning_mask, result_token_positions):
    value, _ = _run(states, reasoning_mask, result_token_positions)
    return np.asarray(value, dtype=np.float32)

